# revision 5
# baseline (speedup 1.0000x reference)
"""Trainium2 Bass kernel for CustomLossWithCovariance.

loss = abs(logdet(sigma) + mean_b[(p_b - t_b)^T sigma^{-1} (p_b - t_b)])

The device only ever needs sufficient statistics of d = pred - targ:

* sigma == c*I (the case this problem instantiates — setup_inputs builds
  sigma = (SIGMA_INIT + EPSILON) * eye(3)): the quadratic form reduces
  EXACTLY to ||d||^2 / c, so the device computes one scalar per
  partition: sum of d_i^2 over the whole shard (fast path,
  build_sumsq_kernel).  This is an algebraic identity, not an
  approximation — sigma_inv's off-diagonals are exact zeros.
* general sigma: the 3x3 Gram matrix G = sum_b d_b d_b^T suffices
  (mean_mahalanobis = <sigma_inv, G> / B); the device computes
  per-core partial pair-sums of G (build_gram_kernel_raw).

Host finishes with the tiny 3x3 algebra in float64.

Sharding: data-parallel over the batch across 8 NeuronCores; each core
streams a contiguous [B/8, 3] shard (24 MiB of f32), so the kernel is
HBM-bandwidth-bound (~60-64 us of streaming per core).  Host-side prep
re-packs each shard partition-major so every DMA descriptor is one
24 KiB contiguous run per partition.

Fast-path device kernel (raw Bacc, manual semaphores): per chunk of
K tiles ([128, K*1536] f32 = pred|targ interleaved per tile):
  - SP:  one dma_start per chunk (128 descriptors x K*6144 B)
  - DVE: one in-place tensor_tensor d = pred - targ  (unit-stride)
  - ACT: one Square activation with accum_out -> per-partition sum d^2
The final chunks are deliberately small (tiles [.., 2, 1, 1]) so the
post-stream compute tail is ~2 us instead of ~8.
"""

import numpy as np

import concourse.bass as bass
import concourse.bacc as bacc
import concourse.mybir as mybir
from concourse import tile
from concourse.bass_utils import run_bass_kernel_spmd

N_CORES = 8
B_FULL = 8388608
P = 128

_PAIRS = [(0, 1), (0, 2), (1, 2)]

# ---------------------------------------------------------------------------
# Walrus flag scoping: the NEFF postamble serially resets every semaphore in
# the allocatable range (256 by default, ~51 EVENT_SEMAPHORE instructions per
# engine ~= 7 us inside the measured execution window).  The fast-path kernel
# uses ~15 semaphores, so cap the range while compiling it.
# ---------------------------------------------------------------------------

_WALRUS_EXTRA_ARGS: list[str] = []

import concourse.bass_utils as _bu

_orig_get_walrus_args = _bu.get_walrus_args


def _patched_get_walrus_args(*args, **kwargs):
    return _orig_get_walrus_args(*args, **kwargs) + list(_WALRUS_EXTRA_ARGS)


_bu.get_walrus_args = _patched_get_walrus_args


class _walrus_flags:
    def __init__(self, *flags: str):
        self.flags = list(flags)

    def __enter__(self):
        _WALRUS_EXTRA_ARGS.extend(self.flags)

    def __exit__(self, *exc):
        for f in self.flags:
            _WALRUS_EXTRA_ARGS.remove(f)

# ---------------------------------------------------------------------------
# Fast path: sigma = c * I  ->  device computes sum over shard of d_i^2.
# ---------------------------------------------------------------------------

# Tiles of r rows per partition; m = 3*r f32 per tile per partition.
SS_R = 256
SS_M = 3 * SS_R                      # 768 f32 = 3 KiB
SS_TILES = 32                        # per core: 32*128*256 = 1,048,576 rows
# Chunk sizes (in tiles).  Bulk chunks of 4 amortize DMA/instruction
# overhead; the trailing 2/1/1 keep the post-stream compute tail short.
SS_CHUNKS = [4, 4, 4, 4, 4, 4, 4, 2, 1, 1]
SS_SLOTS = 7                         # ring slots, each sized for K=4


def build_sumsq_kernel(n_rows: int):
    """Per-core module: in pt [P, SS_TILES*2*SS_M] f32 (per partition, per
    tile: SS_M f32 of pred then SS_M f32 of targ, contiguous); out
    sums [P, n_chunks] f32 where col c = sum over chunk c of (p - t)^2.
    """
    assert n_rows == P * SS_R * SS_TILES
    assert sum(SS_CHUNKS) == SS_TILES
    n_chunks = len(SS_CHUNKS)
    kmax = max(SS_CHUNKS)
    m2 = 2 * SS_M                     # f32 per tile per partition (pred+targ)
    f32 = mybir.dt.float32

    nc = bacc.Bacc("TRN2", target_bir_lowering=False, debug=False)
    pt = nc.dram_tensor("pt", [P, SS_TILES * m2], f32, kind="ExternalInput")
    out = nc.dram_tensor("sums", [P, n_chunks], f32, kind="ExternalOutput")

    ring = nc.alloc_sbuf_tensor("ring", [P, SS_SLOTS * kmax * m2], f32).ap()
    acc = nc.alloc_sbuf_tensor("acc", [P, n_chunks], f32).ap()
    # Dead stores for the Square activations (2 rotating buffers; ACT is
    # serial so program order already fences reuse).
    scr = [nc.alloc_sbuf_tensor(f"scr{i}", [P, kmax * SS_M], f32).ap()
           for i in range(2)]

    dma_sems = [nc.alloc_semaphore(f"dma_sem{i}") for i in range(SS_SLOTS)]
    dve_sem = nc.alloc_semaphore("dve_sem")
    act_sem = nc.alloc_semaphore("act_sem")
    out_sem = nc.alloc_semaphore("out_sem")

    # chunk -> (tile offset, K)
    offs = []
    o = 0
    for k in SS_CHUNKS:
        offs.append((o, k))
        o += k

    def slot_ap(c: int):
        s = (c % SS_SLOTS) * kmax * m2
        k = SS_CHUNKS[c]
        return ring[:, s : s + k * m2]

    import contextlib

    @contextlib.contextmanager
    def _block():
        # Skip the exit-time all-engine sem barrier: outputs are already
        # fenced by the sequencer's out_sem wait, and the NEFF postamble
        # has its own barrier.
        with nc.Block(no_gpsimd_drain=True) as blk:
            try:
                yield blk
            finally:
                nc.all_engine_barrier = lambda **kw: None
        del nc.all_engine_barrier

    with _block() as block:

        @block.sync
        def _(sync):
            for c, (o, k) in enumerate(offs):
                if c >= SS_SLOTS:
                    # ACT is the last reader of the slot's previous chunk
                    sync.wait_ge(act_sem, c - SS_SLOTS + 1)
                sync.dma_start(
                    out=slot_ap(c), in_=pt[:, o * m2 : (o + k) * m2]
                ).then_inc(dma_sems[c % SS_SLOTS], 16)
            # Flush all but the last col while the tail chunks stream; the
            # final flush re-sends col n-2 (identical bytes) because a
            # single-column DRAM slice would be a non-contiguous AP.
            sync.wait_ge(act_sem, n_chunks - 1)
            sync.dma_start(
                out=out[:, 0 : n_chunks - 1], in_=acc[:, 0 : n_chunks - 1]
            ).then_inc(out_sem, 16)
            sync.wait_ge(act_sem, n_chunks)
            sync.dma_start(
                out=out[:, n_chunks - 2 : n_chunks],
                in_=acc[:, n_chunks - 2 : n_chunks],
            ).then_inc(out_sem, 16)
            sync.wait_ge(out_sem, 32)

        @block.vector
        def _(vector):
            for c, (o, k) in enumerate(offs):
                b = slot_ap(c)
                # [k, SS_M] view of the pred / targ halves of each tile
                pred = b.rearrange("p (t w m) -> p t w m", t=k, w=2)[:, :, 0, :]
                targ = b.rearrange("p (t w m) -> p t w m", t=k, w=2)[:, :, 1, :]
                vector.wait_ge(dma_sems[c % SS_SLOTS], 16 * (c // SS_SLOTS + 1))
                vector.tensor_tensor(
                    out=pred, in0=pred, in1=targ, op=mybir.AluOpType.subtract
                ).then_inc(dve_sem, 1)

        @block.scalar
        def _(scalar):
            for c, (o, k) in enumerate(offs):
                b = slot_ap(c)
                d = b.rearrange("p (t w m) -> p t w m", t=k, w=2)[:, :, 0, :]
                scalar.wait_ge(dve_sem, c + 1)
                scalar.activation(
                    out=scr[c % 2][:, : k * SS_M].rearrange(
                        "p (t m) -> p t m", t=k
                    ),
                    in_=d,
                    func=mybir.ActivationFunctionType.Square,
                    accum_out=acc[:, c : c + 1],
                ).then_inc(act_sem, 1)

    nc.compile()
    return nc


def _pack_shard(pred: np.ndarray, targ: np.ndarray) -> np.ndarray:
    """[n_rows, 3] pred/targ -> [P, SS_TILES, 2, SS_M] partition-major
    interleave so each (partition, tile) reads 6 KiB contiguous."""
    n_rows = pred.shape[0]
    assert n_rows == P * SS_TILES * SS_R
    arr = np.empty((P, SS_TILES, 2, SS_M), dtype=np.float32)
    arr[:, :, 0, :] = pred.reshape(P, SS_TILES, SS_M)
    arr[:, :, 1, :] = targ.reshape(P, SS_TILES, SS_M)
    return arr.reshape(P, SS_TILES * 2 * SS_M)


def run_device_sumsq(predictions: np.ndarray, targets: np.ndarray,
                     **run_kwargs):
    """Shard over N_CORES, run fast-path kernel, return per-core sums."""
    b = predictions.shape[0]
    assert b % N_CORES == 0
    n_rows = b // N_CORES
    nc = _get_nc_sumsq(n_rows)
    preds = np.ascontiguousarray(predictions, dtype=np.float32).reshape(
        N_CORES, n_rows, 3
    )
    targs = np.ascontiguousarray(targets, dtype=np.float32).reshape(
        N_CORES, n_rows, 3
    )
    in_maps = [
        {"pt": _pack_shard(preds[c], targs[c])} for c in range(N_CORES)
    ]
    with _walrus_flags("--max-sem-num=64"):
        res = run_bass_kernel_spmd(nc, in_maps, list(range(N_CORES)), **run_kwargs)
    sums = np.stack([r["sums"] for r in res.results])
    return sums, res


def _sigma_is_scalar_identity(sigma64: np.ndarray) -> bool:
    d = np.diag(sigma64)
    return (
        sigma64.shape == (3, 3)
        and np.count_nonzero(sigma64 - np.diag(d)) == 0
        and d[0] == d[1] == d[2]
        and d[0] > 0
    )


# ---------------------------------------------------------------------------
# General path: full 3x3 Gram matrix (kept from the baseline kernel).
# ---------------------------------------------------------------------------


def build_gram_kernel(n_rows: int, n_tiles: int, use_act: bool = True):
    """Build the per-core Bass module.

    Input: pt [2, n_rows, 3] f32 (pred stacked with targ)
    Output: partials [128, 6 * n_tiles] f32
        col t*3+i            : sum over this tile/partition of d_i^2
        col 3*n_tiles + t*3+k: sum of d_i*d_j for pair k in _PAIRS
    """
    assert n_rows % (P * n_tiles) == 0
    r = n_rows // (P * n_tiles)  # rows per partition per tile
    m = 3 * r                    # flat f32 elements per partition per tile
    f32 = mybir.dt.float32

    nc = bacc.Bacc("TRN2", target_bir_lowering=False, debug=False)
    pt = nc.dram_tensor("pt", [2, n_rows, 3], f32, kind="ExternalInput")
    out = nc.dram_tensor("partials", [P, 6 * n_tiles], f32, kind="ExternalOutput")

    pt_v = pt[:].rearrange("w (t p r) c -> t p w (r c)", t=n_tiles, p=P)

    with tile.TileContext(nc) as tc:
        with (
            tc.tile_pool(name="io", bufs=3) as io_pool,
            tc.tile_pool(name="dve_scr", bufs=2) as dve_scr,
            tc.tile_pool(name="act_scr", bufs=2) as act_scr,
            tc.tile_pool(name="acc", bufs=1) as acc_pool,
        ):
            acc_sq = acc_pool.tile([P, 3 * n_tiles], f32)
            acc_cr = acc_pool.tile([P, 3 * n_tiles], f32)

            for t in range(n_tiles):
                buf = io_pool.tile([P, 2 * m], f32, tag="buf")
                nc.sync.dma_start(
                    out=buf[:].rearrange("p (w m) -> p w m", w=2),
                    in_=pt_v[t],
                )

                nc.vector.tensor_tensor(
                    out=buf[:, 0:m],
                    in0=buf[:, 0:m],
                    in1=buf[:, m : 2 * m],
                    op=mybir.AluOpType.subtract,
                )
                d3 = buf[:, 0:m].rearrange("p (r c) -> p c r", c=3)

                if use_act:
                    for i in range(3):
                        sq = act_scr.tile([P, r], f32, tag="sq")
                        nc.scalar.activation(
                            out=sq[:],
                            in_=d3[:, i, :],
                            func=mybir.ActivationFunctionType.Square,
                            accum_out=acc_sq[:, t * 3 + i : t * 3 + i + 1],
                        )
                else:
                    for i in range(3):
                        sq = dve_scr.tile([P, r], f32, tag="pr")
                        nc.vector.scalar_tensor_tensor(
                            out=sq[:],
                            in0=d3[:, i, :],
                            scalar=1.0,
                            in1=d3[:, i, :],
                            op0=mybir.AluOpType.mult,
                            op1=mybir.AluOpType.mult,
                            accum_out=acc_sq[:, t * 3 + i : t * 3 + i + 1],
                        )
                for k, (i, j) in enumerate(_PAIRS):
                    pr = dve_scr.tile([P, r], f32, tag="pr")
                    nc.vector.scalar_tensor_tensor(
                        out=pr[:],
                        in0=d3[:, i, :],
                        scalar=1.0,
                        in1=d3[:, j, :],
                        op0=mybir.AluOpType.mult,
                        op1=mybir.AluOpType.mult,
                        accum_out=acc_cr[:, t * 3 + k : t * 3 + k + 1],
                    )

            nc.sync.dma_start(out=out[:, 0 : 3 * n_tiles], in_=acc_sq[:])
            nc.sync.dma_start(out=out[:, 3 * n_tiles : 6 * n_tiles], in_=acc_cr[:])

    nc.compile()
    return nc


_NC_CACHE: dict[tuple, object] = {}


def _get_nc_sumsq(n_rows: int):
    key = ("sumsq", n_rows)
    if key not in _NC_CACHE:
        _NC_CACHE[key] = build_sumsq_kernel(n_rows)
    return _NC_CACHE[key]


def _get_nc(n_rows: int, n_tiles: int, use_act: bool = True):
    key = (n_rows, n_tiles, use_act)
    if key not in _NC_CACHE:
        _NC_CACHE[key] = build_gram_kernel(n_rows, n_tiles, use_act)
    return _NC_CACHE[key]


def gram_from_partials(partials: np.ndarray, n_tiles: int | None = None) -> np.ndarray:
    """[..., 128, 6*slots] partials -> full 3x3 Gram matrix (float64)."""
    slots = partials.shape[-1] // 6
    s = partials.astype(np.float64).reshape(-1, 6 * slots).sum(axis=0)
    sq = s[: 3 * slots].reshape(slots, 3).sum(axis=0)
    cr = s[3 * slots :].reshape(slots, 3).sum(axis=0)
    g = np.empty((3, 3), dtype=np.float64)
    g[0, 0], g[1, 1], g[2, 2] = sq
    for k, (i, j) in enumerate(_PAIRS):
        g[i, j] = g[j, i] = cr[k]
    return g


def run_device_partials(predictions: np.ndarray, targets: np.ndarray,
                        n_tiles: int = 4, use_act: bool = True,
                        **run_kwargs):
    """Shard over N_CORES, run Gram kernel, return per-core partials."""
    b = predictions.shape[0]
    assert b % N_CORES == 0
    n_rows = b // N_CORES
    nc = _get_nc(n_rows, n_tiles, use_act)
    preds = np.ascontiguousarray(predictions, dtype=np.float32).reshape(
        N_CORES, n_rows, 3
    )
    targs = np.ascontiguousarray(targets, dtype=np.float32).reshape(
        N_CORES, n_rows, 3
    )
    in_maps = [
        {"pt": np.stack([preds[c], targs[c]])} for c in range(N_CORES)
    ]
    res = run_bass_kernel_spmd(nc, in_maps, list(range(N_CORES)), **run_kwargs)
    partials = np.stack([r["partials"] for r in res.results])
    return partials, res


# ---------------------------------------------------------------------------
# Entry point
# ---------------------------------------------------------------------------


def kernel(predictions: np.ndarray, targets: np.ndarray, sigma: np.ndarray) -> np.ndarray:
    predictions = np.asarray(predictions, dtype=np.float32)
    targets = np.asarray(targets, dtype=np.float32)
    sigma64 = np.asarray(sigma, dtype=np.float64)

    _, logdet = np.linalg.slogdet(sigma64)
    n = predictions.shape[0]

    if _sigma_is_scalar_identity(sigma64) and n == B_FULL:
        sums, _ = run_device_sumsq(predictions, targets)
        total = float(sums.astype(np.float64).sum())
        mean_mahal = total / (sigma64[0, 0] * n)
    else:
        partials, _ = run_device_partials(predictions, targets, n_tiles=32)
        g = gram_from_partials(partials)
        sigma_inv = np.linalg.inv(sigma64)
        mean_mahal = float((sigma_inv * g).sum()) / n

    loss = abs(logdet + mean_mahal)
    return np.float32(loss)


# revision 6
# speedup vs baseline: 1.0103x; 1.0103x over previous
"""Trainium2 Bass kernel for CustomLossWithCovariance.

loss = abs(logdet(sigma) + mean_b[(p_b - t_b)^T sigma^{-1} (p_b - t_b)])

The device only ever needs sufficient statistics of d = pred - targ:

* sigma == c*I (the case this problem instantiates — setup_inputs builds
  sigma = (SIGMA_INIT + EPSILON) * eye(3)): the quadratic form reduces
  EXACTLY to ||d||^2 / c, so the device computes one scalar per
  partition: sum of d_i^2 over the whole shard (fast path,
  build_sumsq_kernel).  This is an algebraic identity, not an
  approximation — sigma_inv's off-diagonals are exact zeros.
* general sigma: the 3x3 Gram matrix G = sum_b d_b d_b^T suffices
  (mean_mahalanobis = <sigma_inv, G> / B); the device computes
  per-core partial pair-sums of G (build_gram_kernel_raw).

Host finishes with the tiny 3x3 algebra in float64.

Sharding: data-parallel over the batch across 8 NeuronCores; each core
streams a contiguous [B/8, 3] shard (24 MiB of f32), so the kernel is
HBM-bandwidth-bound (~60-64 us of streaming per core).  Host-side prep
re-packs each shard partition-major so every DMA descriptor is one
24 KiB contiguous run per partition.

Fast-path device kernel (raw Bacc, manual semaphores): per chunk of
K tiles ([128, K*1536] f32 = pred|targ interleaved per tile):
  - SP:  one dma_start per chunk (128 descriptors x K*6144 B)
  - DVE: one in-place tensor_tensor d = pred - targ  (unit-stride)
  - ACT: one Square activation with accum_out -> per-partition sum d^2
The final chunks are deliberately small (tiles [.., 2, 1, 1]) so the
post-stream compute tail is ~2 us instead of ~8.
"""

import numpy as np

import concourse.bass as bass
import concourse.bacc as bacc
import concourse.mybir as mybir
from concourse import tile
from concourse.bass_utils import run_bass_kernel_spmd

N_CORES = 8
B_FULL = 8388608
P = 128

_PAIRS = [(0, 1), (0, 2), (1, 2)]

# ---------------------------------------------------------------------------
# Walrus flag scoping: the NEFF postamble serially resets every semaphore in
# the allocatable range (256 by default, ~51 EVENT_SEMAPHORE instructions per
# engine ~= 7 us inside the measured execution window).  The fast-path kernel
# uses ~15 semaphores, so cap the range while compiling it.
# ---------------------------------------------------------------------------

_WALRUS_EXTRA_ARGS: list[str] = []

import concourse.bass_utils as _bu

_orig_get_walrus_args = _bu.get_walrus_args


def _patched_get_walrus_args(*args, **kwargs):
    return _orig_get_walrus_args(*args, **kwargs) + list(_WALRUS_EXTRA_ARGS)


_bu.get_walrus_args = _patched_get_walrus_args


class _walrus_flags:
    def __init__(self, *flags: str):
        self.flags = list(flags)

    def __enter__(self):
        _WALRUS_EXTRA_ARGS.extend(self.flags)

    def __exit__(self, *exc):
        for f in self.flags:
            _WALRUS_EXTRA_ARGS.remove(f)

# ---------------------------------------------------------------------------
# Fast path: sigma = c * I  ->  device computes sum over shard of d_i^2.
# ---------------------------------------------------------------------------

# Tiles of r rows per partition; m = 3*r f32 per tile per partition.
SS_R = 256
SS_M = 3 * SS_R                      # 768 f32 = 3 KiB
SS_TILES = 32                        # per core: 32*128*256 = 1,048,576 rows
# Chunk sizes (in tiles).  Bulk chunks of 4 amortize DMA/instruction
# overhead; the trailing 2/1/1 keep the post-stream compute tail short.
SS_CHUNKS = [4, 4, 4, 4, 4, 4, 4, 2, 1, 1]
SS_SLOTS = 7                         # ring slots, each sized for K=4


def build_sumsq_kernel(n_rows: int):
    """Per-core module: in pt [P, SS_TILES*2*SS_M] f32 (per partition, per
    tile: SS_M f32 of pred then SS_M f32 of targ, contiguous); out
    sums [P, n_chunks] f32 where col c = sum over chunk c of (p - t)^2.
    """
    assert n_rows == P * SS_R * SS_TILES
    assert sum(SS_CHUNKS) == SS_TILES
    n_chunks = len(SS_CHUNKS)
    kmax = max(SS_CHUNKS)
    m2 = 2 * SS_M                     # f32 per tile per partition (pred+targ)
    f32 = mybir.dt.float32

    nc = bacc.Bacc("TRN2", target_bir_lowering=False, debug=False)
    pt = nc.dram_tensor("pt", [P, SS_TILES * m2], f32, kind="ExternalInput")
    out = nc.dram_tensor("sums", [P, n_chunks], f32, kind="ExternalOutput")

    ring = nc.alloc_sbuf_tensor("ring", [P, SS_SLOTS * kmax * m2], f32).ap()
    acc = nc.alloc_sbuf_tensor("acc", [P, n_chunks], f32).ap()
    # Dead stores for the Square activations (2 rotating buffers; ACT is
    # serial so program order already fences reuse).
    scr = [nc.alloc_sbuf_tensor(f"scr{i}", [P, kmax * SS_M], f32).ap()
           for i in range(2)]

    dma_sems = [nc.alloc_semaphore(f"dma_sem{i}") for i in range(SS_SLOTS)]
    dve_sem = nc.alloc_semaphore("dve_sem")
    act_sem = nc.alloc_semaphore("act_sem")
    out_sem = nc.alloc_semaphore("out_sem")

    # chunk -> (tile offset, K)
    offs = []
    o = 0
    for k in SS_CHUNKS:
        offs.append((o, k))
        o += k

    def slot_ap(c: int):
        s = (c % SS_SLOTS) * kmax * m2
        k = SS_CHUNKS[c]
        return ring[:, s : s + k * m2]

    import contextlib

    @contextlib.contextmanager
    def _block():
        # Skip the exit-time all-engine sem barrier: outputs are already
        # fenced by the sequencer's out_sem wait, and the NEFF postamble
        # has its own barrier.
        with nc.Block(no_gpsimd_drain=True) as blk:
            try:
                yield blk
            finally:
                nc.all_engine_barrier = lambda **kw: None
        del nc.all_engine_barrier

    with _block() as block:

        @block.sync
        def _(sync):
            for c, (o, k) in enumerate(offs):
                if c >= SS_SLOTS:
                    # ACT is the last reader of the slot's previous chunk
                    sync.wait_ge(act_sem, c - SS_SLOTS + 1)
                sync.dma_start(
                    out=slot_ap(c), in_=pt[:, o * m2 : (o + k) * m2]
                ).then_inc(dma_sems[c % SS_SLOTS], 16)
            # Flush all but the last col while the tail chunks stream; the
            # final flush re-sends col n-2 (identical bytes) because a
            # single-column DRAM slice would be a non-contiguous AP.
            sync.wait_ge(act_sem, n_chunks - 1)
            sync.dma_start(
                out=out[:, 0 : n_chunks - 1], in_=acc[:, 0 : n_chunks - 1]
            ).then_inc(out_sem, 16)
            sync.wait_ge(act_sem, n_chunks)
            sync.dma_start(
                out=out[:, n_chunks - 2 : n_chunks],
                in_=acc[:, n_chunks - 2 : n_chunks],
            ).then_inc(out_sem, 16)
            sync.wait_ge(out_sem, 32)

        @block.vector
        def _(vector):
            for c, (o, k) in enumerate(offs):
                b = slot_ap(c)
                # [k, SS_M] view of the pred / targ halves of each tile
                pred = b.rearrange("p (t w m) -> p t w m", t=k, w=2)[:, :, 0, :]
                targ = b.rearrange("p (t w m) -> p t w m", t=k, w=2)[:, :, 1, :]
                vector.wait_ge(dma_sems[c % SS_SLOTS], 16 * (c // SS_SLOTS + 1))
                vector.tensor_tensor(
                    out=pred, in0=pred, in1=targ, op=mybir.AluOpType.subtract
                ).then_inc(dve_sem, 1)

        @block.scalar
        def _(scalar):
            for c, (o, k) in enumerate(offs):
                b = slot_ap(c)
                d = b.rearrange("p (t w m) -> p t w m", t=k, w=2)[:, :, 0, :]
                scalar.wait_ge(dve_sem, c + 1)
                scalar.activation(
                    out=scr[c % 2][:, : k * SS_M].rearrange(
                        "p (t m) -> p t m", t=k
                    ),
                    in_=d,
                    func=mybir.ActivationFunctionType.Square,
                    accum_out=acc[:, c : c + 1],
                ).then_inc(act_sem, 1)

    nc.compile()
    return nc


def _pack_shard(pred: np.ndarray, targ: np.ndarray) -> np.ndarray:
    """[n_rows, 3] pred/targ -> [P, SS_TILES, 2, SS_M] partition-major
    interleave so each (partition, tile) reads 6 KiB contiguous."""
    n_rows = pred.shape[0]
    assert n_rows == P * SS_TILES * SS_R
    arr = np.empty((P, SS_TILES, 2, SS_M), dtype=np.float32)
    arr[:, :, 0, :] = pred.reshape(P, SS_TILES, SS_M)
    arr[:, :, 1, :] = targ.reshape(P, SS_TILES, SS_M)
    return arr.reshape(P, SS_TILES * 2 * SS_M)


def run_device_sumsq(predictions: np.ndarray, targets: np.ndarray,
                     **run_kwargs):
    """Shard over N_CORES, run fast-path kernel, return per-core sums."""
    b = predictions.shape[0]
    assert b % N_CORES == 0
    n_rows = b // N_CORES
    nc = _get_nc_sumsq(n_rows)
    preds = np.ascontiguousarray(predictions, dtype=np.float32).reshape(
        N_CORES, n_rows, 3
    )
    targs = np.ascontiguousarray(targets, dtype=np.float32).reshape(
        N_CORES, n_rows, 3
    )
    in_maps = [
        {"pt": _pack_shard(preds[c], targs[c])} for c in range(N_CORES)
    ]
    res = run_bass_kernel_spmd(nc, in_maps, list(range(N_CORES)), **run_kwargs)
    sums = np.stack([r["sums"] for r in res.results])
    return sums, res


def _sigma_is_scalar_identity(sigma64: np.ndarray) -> bool:
    d = np.diag(sigma64)
    return (
        sigma64.shape == (3, 3)
        and np.count_nonzero(sigma64 - np.diag(d)) == 0
        and d[0] == d[1] == d[2]
        and d[0] > 0
    )


# ---------------------------------------------------------------------------
# General path: full 3x3 Gram matrix (kept from the baseline kernel).
# ---------------------------------------------------------------------------


def build_gram_kernel(n_rows: int, n_tiles: int, use_act: bool = True):
    """Build the per-core Bass module.

    Input: pt [2, n_rows, 3] f32 (pred stacked with targ)
    Output: partials [128, 6 * n_tiles] f32
        col t*3+i            : sum over this tile/partition of d_i^2
        col 3*n_tiles + t*3+k: sum of d_i*d_j for pair k in _PAIRS
    """
    assert n_rows % (P * n_tiles) == 0
    r = n_rows // (P * n_tiles)  # rows per partition per tile
    m = 3 * r                    # flat f32 elements per partition per tile
    f32 = mybir.dt.float32

    nc = bacc.Bacc("TRN2", target_bir_lowering=False, debug=False)
    pt = nc.dram_tensor("pt", [2, n_rows, 3], f32, kind="ExternalInput")
    out = nc.dram_tensor("partials", [P, 6 * n_tiles], f32, kind="ExternalOutput")

    pt_v = pt[:].rearrange("w (t p r) c -> t p w (r c)", t=n_tiles, p=P)

    with tile.TileContext(nc) as tc:
        with (
            tc.tile_pool(name="io", bufs=3) as io_pool,
            tc.tile_pool(name="dve_scr", bufs=2) as dve_scr,
            tc.tile_pool(name="act_scr", bufs=2) as act_scr,
            tc.tile_pool(name="acc", bufs=1) as acc_pool,
        ):
            acc_sq = acc_pool.tile([P, 3 * n_tiles], f32)
            acc_cr = acc_pool.tile([P, 3 * n_tiles], f32)

            for t in range(n_tiles):
                buf = io_pool.tile([P, 2 * m], f32, tag="buf")
                nc.sync.dma_start(
                    out=buf[:].rearrange("p (w m) -> p w m", w=2),
                    in_=pt_v[t],
                )

                nc.vector.tensor_tensor(
                    out=buf[:, 0:m],
                    in0=buf[:, 0:m],
                    in1=buf[:, m : 2 * m],
                    op=mybir.AluOpType.subtract,
                )
                d3 = buf[:, 0:m].rearrange("p (r c) -> p c r", c=3)

                if use_act:
                    for i in range(3):
                        sq = act_scr.tile([P, r], f32, tag="sq")
                        nc.scalar.activation(
                            out=sq[:],
                            in_=d3[:, i, :],
                            func=mybir.ActivationFunctionType.Square,
                            accum_out=acc_sq[:, t * 3 + i : t * 3 + i + 1],
                        )
                else:
                    for i in range(3):
                        sq = dve_scr.tile([P, r], f32, tag="pr")
                        nc.vector.scalar_tensor_tensor(
                            out=sq[:],
                            in0=d3[:, i, :],
                            scalar=1.0,
                            in1=d3[:, i, :],
                            op0=mybir.AluOpType.mult,
                            op1=mybir.AluOpType.mult,
                            accum_out=acc_sq[:, t * 3 + i : t * 3 + i + 1],
                        )
                for k, (i, j) in enumerate(_PAIRS):
                    pr = dve_scr.tile([P, r], f32, tag="pr")
                    nc.vector.scalar_tensor_tensor(
                        out=pr[:],
                        in0=d3[:, i, :],
                        scalar=1.0,
                        in1=d3[:, j, :],
                        op0=mybir.AluOpType.mult,
                        op1=mybir.AluOpType.mult,
                        accum_out=acc_cr[:, t * 3 + k : t * 3 + k + 1],
                    )

            nc.sync.dma_start(out=out[:, 0 : 3 * n_tiles], in_=acc_sq[:])
            nc.sync.dma_start(out=out[:, 3 * n_tiles : 6 * n_tiles], in_=acc_cr[:])

    nc.compile()
    return nc


_NC_CACHE: dict[tuple, object] = {}


def _get_nc_sumsq(n_rows: int):
    key = ("sumsq", n_rows)
    if key not in _NC_CACHE:
        _NC_CACHE[key] = build_sumsq_kernel(n_rows)
    return _NC_CACHE[key]


def _get_nc(n_rows: int, n_tiles: int, use_act: bool = True):
    key = (n_rows, n_tiles, use_act)
    if key not in _NC_CACHE:
        _NC_CACHE[key] = build_gram_kernel(n_rows, n_tiles, use_act)
    return _NC_CACHE[key]


def gram_from_partials(partials: np.ndarray, n_tiles: int | None = None) -> np.ndarray:
    """[..., 128, 6*slots] partials -> full 3x3 Gram matrix (float64)."""
    slots = partials.shape[-1] // 6
    s = partials.astype(np.float64).reshape(-1, 6 * slots).sum(axis=0)
    sq = s[: 3 * slots].reshape(slots, 3).sum(axis=0)
    cr = s[3 * slots :].reshape(slots, 3).sum(axis=0)
    g = np.empty((3, 3), dtype=np.float64)
    g[0, 0], g[1, 1], g[2, 2] = sq
    for k, (i, j) in enumerate(_PAIRS):
        g[i, j] = g[j, i] = cr[k]
    return g


def run_device_partials(predictions: np.ndarray, targets: np.ndarray,
                        n_tiles: int = 4, use_act: bool = True,
                        **run_kwargs):
    """Shard over N_CORES, run Gram kernel, return per-core partials."""
    b = predictions.shape[0]
    assert b % N_CORES == 0
    n_rows = b // N_CORES
    nc = _get_nc(n_rows, n_tiles, use_act)
    preds = np.ascontiguousarray(predictions, dtype=np.float32).reshape(
        N_CORES, n_rows, 3
    )
    targs = np.ascontiguousarray(targets, dtype=np.float32).reshape(
        N_CORES, n_rows, 3
    )
    in_maps = [
        {"pt": np.stack([preds[c], targs[c]])} for c in range(N_CORES)
    ]
    res = run_bass_kernel_spmd(nc, in_maps, list(range(N_CORES)), **run_kwargs)
    partials = np.stack([r["partials"] for r in res.results])
    return partials, res


# ---------------------------------------------------------------------------
# Entry point
# ---------------------------------------------------------------------------


def kernel(predictions: np.ndarray, targets: np.ndarray, sigma: np.ndarray) -> np.ndarray:
    predictions = np.asarray(predictions, dtype=np.float32)
    targets = np.asarray(targets, dtype=np.float32)
    sigma64 = np.asarray(sigma, dtype=np.float64)

    _, logdet = np.linalg.slogdet(sigma64)
    n = predictions.shape[0]

    if _sigma_is_scalar_identity(sigma64) and n == B_FULL:
        sums, _ = run_device_sumsq(predictions, targets)
        total = float(sums.astype(np.float64).sum())
        mean_mahal = total / (sigma64[0, 0] * n)
    else:
        partials, _ = run_device_partials(predictions, targets, n_tiles=32)
        g = gram_from_partials(partials)
        sigma_inv = np.linalg.inv(sigma64)
        mean_mahal = float((sigma_inv * g).sum()) / n

    loss = abs(logdet + mean_mahal)
    return np.float32(loss)


# revision 18
# speedup vs baseline: 1.1985x; 1.1862x over previous
"""Trainium2 Bass kernel for CustomLossWithCovariance.

loss = abs(logdet(sigma) + mean_b[(p_b - t_b)^T sigma^{-1} (p_b - t_b)])

The device only ever needs sufficient statistics of d = pred - targ:

* sigma == c*I (the case this problem instantiates — setup_inputs builds
  sigma = (SIGMA_INIT + EPSILON) * eye(3)): the quadratic form reduces
  EXACTLY to ||d||^2 / c, so the device computes one scalar per
  partition: sum of d_i^2 over the whole shard (fast path,
  build_sumsq_kernel).  This is an algebraic identity, not an
  approximation — sigma_inv's off-diagonals are exact zeros.
* general sigma: the 3x3 Gram matrix G = sum_b d_b d_b^T suffices
  (mean_mahalanobis = <sigma_inv, G> / B); the device computes
  per-core partial pair-sums of G (build_gram_kernel_raw).

Host finishes with the tiny 3x3 algebra in float64.

Sharding: data-parallel over the batch across 8 NeuronCores; each core
streams a contiguous [B/8, 3] shard (24 MiB of f32), so the kernel is
HBM-bandwidth-bound (~60-64 us of streaming per core).  Host-side prep
re-packs each shard partition-major so every DMA descriptor is one
24 KiB contiguous run per partition.

Fast-path device kernel (raw Bacc, manual semaphores): per chunk of
K tiles ([128, K*1536] f32 = pred|targ interleaved per tile):
  - SP:  one dma_start per chunk (128 descriptors x K*6144 B)
  - DVE: one in-place tensor_tensor d = pred - targ  (unit-stride)
  - ACT: one Square activation with accum_out -> per-partition sum d^2
The final chunks are deliberately small (tiles [.., 2, 1, 1]) so the
post-stream compute tail is ~2 us instead of ~8.
"""

import numpy as np

import concourse.bass as bass
import concourse.bacc as bacc
import concourse.mybir as mybir
from concourse import tile
from concourse.bass_utils import run_bass_kernel_spmd

N_CORES = 8
B_FULL = 8388608
P = 128

_PAIRS = [(0, 1), (0, 2), (1, 2)]

# ---------------------------------------------------------------------------
# Walrus flag scoping: the NEFF postamble serially resets every semaphore in
# the allocatable range (256 by default, ~51 EVENT_SEMAPHORE instructions per
# engine ~= 7 us inside the measured execution window).  The fast-path kernel
# uses ~15 semaphores, so cap the range while compiling it.
# ---------------------------------------------------------------------------

_WALRUS_EXTRA_ARGS: list[str] = []

import concourse.bass_utils as _bu

_orig_get_walrus_args = _bu.get_walrus_args


def _patched_get_walrus_args(*args, **kwargs):
    return _orig_get_walrus_args(*args, **kwargs) + list(_WALRUS_EXTRA_ARGS)


_bu.get_walrus_args = _patched_get_walrus_args


class _walrus_flags:
    def __init__(self, *flags: str):
        self.flags = list(flags)

    def __enter__(self):
        _WALRUS_EXTRA_ARGS.extend(self.flags)

    def __exit__(self, *exc):
        for f in self.flags:
            _WALRUS_EXTRA_ARGS.remove(f)

# ---------------------------------------------------------------------------
# Fast path (skewed): sigma = c * I  ->  device computes sum of d_i^2.
#
# SDMA engine 15 on these parts runs chronically 5-22% slower than the other
# fifteen (known TRN2 erratum), and a 128-partition DMA assigns descriptor i
# (= partition i) to engine slot i mod 16, so uniform layouts are paced by
# engine 15.  Mitigation: ~12% of the rows are carried by 15-descriptor
# "top-up" DMAs, which fill engine slots 0-14 and never touch engine 15
# (HW-verified: descriptor round-robin restarts at slot 0 per dma_start).
# A 15-descriptor DMA's completion increments its semaphore by 15, not 16.
# ---------------------------------------------------------------------------

SK_R = 64                  # rows per partition per tile
SK_M = 3 * SK_R            # 192 f32 per tile half
SK_M2 = 2 * SK_M           # 384 f32 (pred|targ) per tile per partition
SK_TM = 113                # main tiles per partition (128 partitions)
SK_TT = 16                 # top-up tiles per lane (120 lanes, 8 chunks of 15)
# main chunks in tiles (sum = SK_TM); small trailing chunks keep the
# post-stream tail short
SK_MAIN_CHUNKS = [16, 16, 16, 16, 16, 16, 12, 4, 1]
SK_TOP_CHUNKS = 8          # each: [15 partitions, 16 tiles] on partitions 0-14
SK_MAIN_SLOTS = 4
SK_TOP_SLOTS = 3


def _sk_schedule():
    """Global issue order: (kind, idx). Top-ups interleave into the main
    stream (their DMAs are 15/128 the size of a main chunk); even top-ups'
    subtracts run on GpSimd, odd ones on DVE, so neither engine's backlog
    outlives the stream.  The last two chunks are the small main tails."""
    return [
        ("m", 0), ("t", 0), ("m", 1), ("t", 1), ("m", 2), ("t", 2),
        ("m", 3), ("t", 3), ("m", 4), ("t", 4), ("m", 5), ("t", 5),
        ("m", 6), ("t", 6), ("t", 7), ("m", 7), ("m", 8),
    ]


def _sk_sub_engine(kind: str, idx: int) -> str:
    """Which engine runs the subtract for a chunk.  All on DVE: GpSimd
    tensor ops measurably slowed concurrent SDMA streaming (and their
    completion semaphore does not fence SBUF write drain), so GpSimd is
    kept idle."""
    return "dve"


def build_sumsq_kernel_v3(n_rows: int):
    """Per-core module, engine-15-skewed layout.

    Inputs:
      pt_main [128, SK_TM*SK_M2] f32 — per partition, per tile: SK_M f32 of
          pred then SK_M of targ, contiguous.
      pt_top  [120, SK_TT*SK_M2] f32 — top-up rows; chunk j = rows 15j..15j+15.
    Output: sums [128, n_cols] f32; col layout = one col per chunk in issue
    order; top-up cols only have partitions 15j..15j+15 valid.
    """
    assert n_rows == P * SK_R * SK_TM + 120 * SK_R * SK_TT
    assert sum(SK_MAIN_CHUNKS) == SK_TM
    f32 = mybir.dt.float32
    order = _sk_schedule()
    n_cols = len(order)
    kmax = max(SK_MAIN_CHUNKS)

    nc = bacc.Bacc("TRN2", target_bir_lowering=False, debug=False)
    pt_main = nc.dram_tensor("pt_main", [P, SK_TM * SK_M2], f32,
                             kind="ExternalInput")
    pt_top = nc.dram_tensor("pt_top", [120, SK_TT * SK_M2], f32,
                            kind="ExternalInput")
    out = nc.dram_tensor("sums", [P, n_cols], f32, kind="ExternalOutput")

    mring = nc.alloc_sbuf_tensor("mring", [P, SK_MAIN_SLOTS * kmax * SK_M2],
                                 f32).ap()
    tring = nc.alloc_sbuf_tensor("tring", [P, SK_TOP_SLOTS * SK_TT * SK_M2],
                                 f32).ap()
    acc = nc.alloc_sbuf_tensor("acc", [P, n_cols], f32).ap()
    scr = [nc.alloc_sbuf_tensor(f"scr{i}", [P, kmax * SK_M], f32).ap()
           for i in range(2)]

    mdma_sems = [nc.alloc_semaphore(f"mdma{i}") for i in range(SK_MAIN_SLOTS)]
    tdma_sems = [nc.alloc_semaphore(f"tdma{i}") for i in range(SK_TOP_SLOTS)]
    dve_sem = nc.alloc_semaphore("dve_sem")
    gp_sem = nc.alloc_semaphore("gp_sem")
    act_sem = nc.alloc_semaphore("act_sem")
    out_sem = nc.alloc_semaphore("out_sem")

    # per-chunk subtract engine and that engine's running count at the chunk
    sub_eng = {}
    sub_count = {}
    counts = {"dve": 0, "gp": 0}
    for key in order:
        e = _sk_sub_engine(*key)
        counts[e] += 1
        sub_eng[key] = e
        sub_count[key] = counts[e]

    # Per-chunk geometry -----------------------------------------------------
    moffs = []
    o = 0
    for k in SK_MAIN_CHUNKS:
        moffs.append((o, k)); o += k

    def m_slot(mi):
        s = (mi % SK_MAIN_SLOTS) * kmax * SK_M2
        k = SK_MAIN_CHUNKS[mi]
        return mring[:, s : s + k * SK_M2]

    def t_slot(ti):
        # compute engines require quadrant-aligned partition bases, so all
        # top-up chunks live on partitions 0-14
        s = (ti % SK_TOP_SLOTS) * SK_TT * SK_M2
        return tring[0:15, s : s + SK_TT * SK_M2]

    # issue-order position of each chunk, and per-slot reuse dependencies
    pos = {key: i for i, key in enumerate(order)}

    import contextlib

    @contextlib.contextmanager
    def _block():
        with nc.Block(no_gpsimd_drain=True) as blk:
            try:
                yield blk
            finally:
                nc.all_engine_barrier = lambda **kw: None
        del nc.all_engine_barrier

    with _block() as block:

        @block.sync
        def _(sync):
            for i, (kind, idx) in enumerate(order):
                if kind == "m":
                    if idx >= SK_MAIN_SLOTS:
                        # ACT is the slot's last reader
                        sync.wait_ge(act_sem, pos[("m", idx - SK_MAIN_SLOTS)] + 1)
                    o, k = moffs[idx]
                    sync.dma_start(
                        out=m_slot(idx),
                        in_=pt_main[:, o * SK_M2 : (o + k) * SK_M2],
                    ).then_inc(mdma_sems[idx % SK_MAIN_SLOTS], 16)
                else:
                    if idx >= SK_TOP_SLOTS:
                        sync.wait_ge(act_sem, pos[("t", idx - SK_TOP_SLOTS)] + 1)
                    lo = 15 * idx
                    sync.dma_start(
                        out=t_slot(idx), in_=pt_top[lo : lo + 15, :]
                    ).then_inc(tdma_sems[idx % SK_TOP_SLOTS], 16)
            # Flush all but the last two cols early, then the tail pair.
            sync.wait_ge(act_sem, n_cols - 1)
            sync.dma_start(
                out=out[:, 0 : n_cols - 1], in_=acc[:, 0 : n_cols - 1]
            ).then_inc(out_sem, 16)
            sync.wait_ge(act_sem, n_cols)
            sync.dma_start(
                out=out[:, n_cols - 2 : n_cols], in_=acc[:, n_cols - 2 : n_cols]
            ).then_inc(out_sem, 16)
            sync.wait_ge(out_sem, 32)

        def chunk_halves(kind, idx):
            if kind == "m":
                b = m_slot(idx)
                k = SK_MAIN_CHUNKS[idx]
            else:
                b = t_slot(idx)
                k = SK_TT
            pred = b.rearrange("p (t w m) -> p t w m", t=k, w=2)[:, :, 0, :]
            targ = b.rearrange("p (t w m) -> p t w m", t=k, w=2)[:, :, 1, :]
            return pred, targ

        def emit_sub(eng, done_sem, kind, idx):
            pred, targ = chunk_halves(kind, idx)
            if kind == "m":
                eng.wait_ge(
                    mdma_sems[idx % SK_MAIN_SLOTS],
                    16 * (idx // SK_MAIN_SLOTS + 1),
                )
            else:
                # 15-descriptor DMA: 15 engines -> +15 per completion
                eng.wait_ge(
                    tdma_sems[idx % SK_TOP_SLOTS],
                    15 * (idx // SK_TOP_SLOTS + 1),
                )
            eng.tensor_tensor(
                out=pred, in0=pred, in1=targ, op=mybir.AluOpType.subtract
            ).then_inc(done_sem, 1)

        @block.vector
        def _(vector):
            for key in order:
                if sub_eng[key] == "dve":
                    emit_sub(vector, dve_sem, *key)

        @block.gpsimd
        def _(gpsimd):
            for key in order:
                if sub_eng[key] == "gp":
                    emit_sub(gpsimd, gp_sem, *key)

        @block.scalar
        def _(scalar):
            for i, (kind, idx) in enumerate(order):
                pred, targ = chunk_halves(kind, idx)
                if kind == "m":
                    acc_ap = acc[:, i : i + 1]
                else:
                    acc_ap = acc[0:15, i : i + 1]
                sem = dve_sem if sub_eng[(kind, idx)] == "dve" else gp_sem
                scalar.wait_ge(sem, sub_count[(kind, idx)])
                k = SK_MAIN_CHUNKS[idx] if kind == "m" else SK_TT
                np_ = P if kind == "m" else 15
                scalar.activation(
                    out=scr[i % 2][0:np_, 0 : k * SK_M].rearrange(
                        "p (t m) -> p t m", t=k
                    ),
                    in_=pred,
                    func=mybir.ActivationFunctionType.Square,
                    accum_out=acc_ap,
                ).then_inc(act_sem, 1)

    nc.compile()
    return nc


def _pack_shard_v3(pred: np.ndarray, targ: np.ndarray):
    """[n_rows, 3] -> (pt_main [128, SK_TM*SK_M2], pt_top [120, SK_TT*SK_M2])."""
    n_main = P * SK_TM * SK_R
    pm = np.empty((P, SK_TM, 2, SK_M), dtype=np.float32)
    pm[:, :, 0, :] = pred[:n_main].reshape(P, SK_TM, SK_M)
    pm[:, :, 1, :] = targ[:n_main].reshape(P, SK_TM, SK_M)
    tp = np.empty((120, SK_TT, 2, SK_M), dtype=np.float32)
    tp[:, :, 0, :] = pred[n_main:].reshape(120, SK_TT, SK_M)
    tp[:, :, 1, :] = targ[n_main:].reshape(120, SK_TT, SK_M)
    return pm.reshape(P, SK_TM * SK_M2), tp.reshape(120, SK_TT * SK_M2)


def sumsq_from_cols_v3(sums: np.ndarray) -> float:
    """[n_cores, 128, n_cols] device cols -> total sum of d^2 (float64)."""
    order = _sk_schedule()
    total = 0.0
    for c in range(sums.shape[0]):
        for i, (kind, idx) in enumerate(order):
            col = sums[c, :, i].astype(np.float64)
            if kind == "m":
                total += col.sum()
            else:
                total += col[0:15].sum()
    return total


def run_device_sumsq_v3(predictions: np.ndarray, targets: np.ndarray,
                        **run_kwargs):
    b = predictions.shape[0]
    assert b % N_CORES == 0
    n_rows = b // N_CORES
    nc = _get_nc_sumsq_v3(n_rows)
    preds = np.ascontiguousarray(predictions, dtype=np.float32).reshape(
        N_CORES, n_rows, 3
    )
    targs = np.ascontiguousarray(targets, dtype=np.float32).reshape(
        N_CORES, n_rows, 3
    )
    in_maps = []
    for c in range(N_CORES):
        pm, tp = _pack_shard_v3(preds[c], targs[c])
        in_maps.append({"pt_main": pm, "pt_top": tp})
    res = run_bass_kernel_spmd(nc, in_maps, list(range(N_CORES)), **run_kwargs)
    sums = np.stack([r["sums"] for r in res.results])
    return sums, res


def _get_nc_sumsq_v3(n_rows: int):
    key = ("sumsq_v3", n_rows)
    if key not in _NC_CACHE:
        _NC_CACHE[key] = build_sumsq_kernel_v3(n_rows)
    return _NC_CACHE[key]


# ---------------------------------------------------------------------------
# Fast path (uniform): sigma = c * I  ->  device computes sum of d_i^2.
# ---------------------------------------------------------------------------

# Tiles of r rows per partition; m = 3*r f32 per tile per partition.
SS_R = 256
SS_M = 3 * SS_R                      # 768 f32 = 3 KiB
SS_TILES = 32                        # per core: 32*128*256 = 1,048,576 rows
# Chunk sizes (in tiles).  Bulk chunks of 4 amortize DMA/instruction
# overhead; the trailing 2/1/1 keep the post-stream compute tail short.
SS_CHUNKS = [4, 4, 4, 4, 4, 4, 4, 2, 1, 1]
SS_SLOTS = 7                         # ring slots, each sized for K=4


def build_sumsq_kernel(n_rows: int):
    """Per-core module: in pt [P, SS_TILES*2*SS_M] f32 (per partition, per
    tile: SS_M f32 of pred then SS_M f32 of targ, contiguous); out
    sums [P, n_chunks] f32 where col c = sum over chunk c of (p - t)^2.
    """
    assert n_rows == P * SS_R * SS_TILES
    assert sum(SS_CHUNKS) == SS_TILES
    n_chunks = len(SS_CHUNKS)
    kmax = max(SS_CHUNKS)
    m2 = 2 * SS_M                     # f32 per tile per partition (pred+targ)
    f32 = mybir.dt.float32

    nc = bacc.Bacc("TRN2", target_bir_lowering=False, debug=False)
    pt = nc.dram_tensor("pt", [P, SS_TILES * m2], f32, kind="ExternalInput")
    out = nc.dram_tensor("sums", [P, n_chunks], f32, kind="ExternalOutput")

    ring = nc.alloc_sbuf_tensor("ring", [P, SS_SLOTS * kmax * m2], f32).ap()
    acc = nc.alloc_sbuf_tensor("acc", [P, n_chunks], f32).ap()
    # Dead stores for the Square activations (2 rotating buffers; ACT is
    # serial so program order already fences reuse).
    scr = [nc.alloc_sbuf_tensor(f"scr{i}", [P, kmax * SS_M], f32).ap()
           for i in range(2)]

    dma_sems = [nc.alloc_semaphore(f"dma_sem{i}") for i in range(SS_SLOTS)]
    dve_sem = nc.alloc_semaphore("dve_sem")
    act_sem = nc.alloc_semaphore("act_sem")
    out_sem = nc.alloc_semaphore("out_sem")

    # chunk -> (tile offset, K)
    offs = []
    o = 0
    for k in SS_CHUNKS:
        offs.append((o, k))
        o += k

    def slot_ap(c: int):
        s = (c % SS_SLOTS) * kmax * m2
        k = SS_CHUNKS[c]
        return ring[:, s : s + k * m2]

    import contextlib

    @contextlib.contextmanager
    def _block():
        # Skip the exit-time all-engine sem barrier: outputs are already
        # fenced by the sequencer's out_sem wait, and the NEFF postamble
        # has its own barrier.
        with nc.Block(no_gpsimd_drain=True) as blk:
            try:
                yield blk
            finally:
                nc.all_engine_barrier = lambda **kw: None
        del nc.all_engine_barrier

    with _block() as block:

        @block.sync
        def _(sync):
            for c, (o, k) in enumerate(offs):
                if c >= SS_SLOTS:
                    # ACT is the last reader of the slot's previous chunk
                    sync.wait_ge(act_sem, c - SS_SLOTS + 1)
                sync.dma_start(
                    out=slot_ap(c), in_=pt[:, o * m2 : (o + k) * m2]
                ).then_inc(dma_sems[c % SS_SLOTS], 16)
            # Flush all but the last col while the tail chunks stream; the
            # final flush re-sends col n-2 (identical bytes) because a
            # single-column DRAM slice would be a non-contiguous AP.
            sync.wait_ge(act_sem, n_chunks - 1)
            sync.dma_start(
                out=out[:, 0 : n_chunks - 1], in_=acc[:, 0 : n_chunks - 1]
            ).then_inc(out_sem, 16)
            sync.wait_ge(act_sem, n_chunks)
            sync.dma_start(
                out=out[:, n_chunks - 2 : n_chunks],
                in_=acc[:, n_chunks - 2 : n_chunks],
            ).then_inc(out_sem, 16)
            sync.wait_ge(out_sem, 32)

        @block.vector
        def _(vector):
            for c, (o, k) in enumerate(offs):
                b = slot_ap(c)
                # [k, SS_M] view of the pred / targ halves of each tile
                pred = b.rearrange("p (t w m) -> p t w m", t=k, w=2)[:, :, 0, :]
                targ = b.rearrange("p (t w m) -> p t w m", t=k, w=2)[:, :, 1, :]
                vector.wait_ge(dma_sems[c % SS_SLOTS], 16 * (c // SS_SLOTS + 1))
                vector.tensor_tensor(
                    out=pred, in0=pred, in1=targ, op=mybir.AluOpType.subtract
                ).then_inc(dve_sem, 1)

        @block.scalar
        def _(scalar):
            for c, (o, k) in enumerate(offs):
                b = slot_ap(c)
                d = b.rearrange("p (t w m) -> p t w m", t=k, w=2)[:, :, 0, :]
                scalar.wait_ge(dve_sem, c + 1)
                scalar.activation(
                    out=scr[c % 2][:, : k * SS_M].rearrange(
                        "p (t m) -> p t m", t=k
                    ),
                    in_=d,
                    func=mybir.ActivationFunctionType.Square,
                    accum_out=acc[:, c : c + 1],
                ).then_inc(act_sem, 1)

    nc.compile()
    return nc


def _pack_shard(pred: np.ndarray, targ: np.ndarray) -> np.ndarray:
    """[n_rows, 3] pred/targ -> [P, SS_TILES, 2, SS_M] partition-major
    interleave so each (partition, tile) reads 6 KiB contiguous."""
    n_rows = pred.shape[0]
    assert n_rows == P * SS_TILES * SS_R
    arr = np.empty((P, SS_TILES, 2, SS_M), dtype=np.float32)
    arr[:, :, 0, :] = pred.reshape(P, SS_TILES, SS_M)
    arr[:, :, 1, :] = targ.reshape(P, SS_TILES, SS_M)
    return arr.reshape(P, SS_TILES * 2 * SS_M)


def run_device_sumsq(predictions: np.ndarray, targets: np.ndarray,
                     **run_kwargs):
    """Shard over N_CORES, run fast-path kernel, return per-core sums."""
    b = predictions.shape[0]
    assert b % N_CORES == 0
    n_rows = b // N_CORES
    nc = _get_nc_sumsq(n_rows)
    preds = np.ascontiguousarray(predictions, dtype=np.float32).reshape(
        N_CORES, n_rows, 3
    )
    targs = np.ascontiguousarray(targets, dtype=np.float32).reshape(
        N_CORES, n_rows, 3
    )
    in_maps = [
        {"pt": _pack_shard(preds[c], targs[c])} for c in range(N_CORES)
    ]
    res = run_bass_kernel_spmd(nc, in_maps, list(range(N_CORES)), **run_kwargs)
    sums = np.stack([r["sums"] for r in res.results])
    return sums, res


def _sigma_is_scalar_identity(sigma64: np.ndarray) -> bool:
    d = np.diag(sigma64)
    return (
        sigma64.shape == (3, 3)
        and np.count_nonzero(sigma64 - np.diag(d)) == 0
        and d[0] == d[1] == d[2]
        and d[0] > 0
    )


# ---------------------------------------------------------------------------
# General path: full 3x3 Gram matrix (kept from the baseline kernel).
# ---------------------------------------------------------------------------


def build_gram_kernel(n_rows: int, n_tiles: int, use_act: bool = True):
    """Build the per-core Bass module.

    Input: pt [2, n_rows, 3] f32 (pred stacked with targ)
    Output: partials [128, 6 * n_tiles] f32
        col t*3+i            : sum over this tile/partition of d_i^2
        col 3*n_tiles + t*3+k: sum of d_i*d_j for pair k in _PAIRS
    """
    assert n_rows % (P * n_tiles) == 0
    r = n_rows // (P * n_tiles)  # rows per partition per tile
    m = 3 * r                    # flat f32 elements per partition per tile
    f32 = mybir.dt.float32

    nc = bacc.Bacc("TRN2", target_bir_lowering=False, debug=False)
    pt = nc.dram_tensor("pt", [2, n_rows, 3], f32, kind="ExternalInput")
    out = nc.dram_tensor("partials", [P, 6 * n_tiles], f32, kind="ExternalOutput")

    pt_v = pt[:].rearrange("w (t p r) c -> t p w (r c)", t=n_tiles, p=P)

    with tile.TileContext(nc) as tc:
        with (
            tc.tile_pool(name="io", bufs=3) as io_pool,
            tc.tile_pool(name="dve_scr", bufs=2) as dve_scr,
            tc.tile_pool(name="act_scr", bufs=2) as act_scr,
            tc.tile_pool(name="acc", bufs=1) as acc_pool,
        ):
            acc_sq = acc_pool.tile([P, 3 * n_tiles], f32)
            acc_cr = acc_pool.tile([P, 3 * n_tiles], f32)

            for t in range(n_tiles):
                buf = io_pool.tile([P, 2 * m], f32, tag="buf")
                nc.sync.dma_start(
                    out=buf[:].rearrange("p (w m) -> p w m", w=2),
                    in_=pt_v[t],
                )

                nc.vector.tensor_tensor(
                    out=buf[:, 0:m],
                    in0=buf[:, 0:m],
                    in1=buf[:, m : 2 * m],
                    op=mybir.AluOpType.subtract,
                )
                d3 = buf[:, 0:m].rearrange("p (r c) -> p c r", c=3)

                if use_act:
                    for i in range(3):
                        sq = act_scr.tile([P, r], f32, tag="sq")
                        nc.scalar.activation(
                            out=sq[:],
                            in_=d3[:, i, :],
                            func=mybir.ActivationFunctionType.Square,
                            accum_out=acc_sq[:, t * 3 + i : t * 3 + i + 1],
                        )
                else:
                    for i in range(3):
                        sq = dve_scr.tile([P, r], f32, tag="pr")
                        nc.vector.scalar_tensor_tensor(
                            out=sq[:],
                            in0=d3[:, i, :],
                            scalar=1.0,
                            in1=d3[:, i, :],
                            op0=mybir.AluOpType.mult,
                            op1=mybir.AluOpType.mult,
                            accum_out=acc_sq[:, t * 3 + i : t * 3 + i + 1],
                        )
                for k, (i, j) in enumerate(_PAIRS):
                    pr = dve_scr.tile([P, r], f32, tag="pr")
                    nc.vector.scalar_tensor_tensor(
                        out=pr[:],
                        in0=d3[:, i, :],
                        scalar=1.0,
                        in1=d3[:, j, :],
                        op0=mybir.AluOpType.mult,
                        op1=mybir.AluOpType.mult,
                        accum_out=acc_cr[:, t * 3 + k : t * 3 + k + 1],
                    )

            nc.sync.dma_start(out=out[:, 0 : 3 * n_tiles], in_=acc_sq[:])
            nc.sync.dma_start(out=out[:, 3 * n_tiles : 6 * n_tiles], in_=acc_cr[:])

    nc.compile()
    return nc


_NC_CACHE: dict[tuple, object] = {}


def _get_nc_sumsq(n_rows: int):
    key = ("sumsq", n_rows)
    if key not in _NC_CACHE:
        _NC_CACHE[key] = build_sumsq_kernel(n_rows)
    return _NC_CACHE[key]


def _get_nc(n_rows: int, n_tiles: int, use_act: bool = True):
    key = (n_rows, n_tiles, use_act)
    if key not in _NC_CACHE:
        _NC_CACHE[key] = build_gram_kernel(n_rows, n_tiles, use_act)
    return _NC_CACHE[key]


def gram_from_partials(partials: np.ndarray, n_tiles: int | None = None) -> np.ndarray:
    """[..., 128, 6*slots] partials -> full 3x3 Gram matrix (float64)."""
    slots = partials.shape[-1] // 6
    s = partials.astype(np.float64).reshape(-1, 6 * slots).sum(axis=0)
    sq = s[: 3 * slots].reshape(slots, 3).sum(axis=0)
    cr = s[3 * slots :].reshape(slots, 3).sum(axis=0)
    g = np.empty((3, 3), dtype=np.float64)
    g[0, 0], g[1, 1], g[2, 2] = sq
    for k, (i, j) in enumerate(_PAIRS):
        g[i, j] = g[j, i] = cr[k]
    return g


def run_device_partials(predictions: np.ndarray, targets: np.ndarray,
                        n_tiles: int = 4, use_act: bool = True,
                        **run_kwargs):
    """Shard over N_CORES, run Gram kernel, return per-core partials."""
    b = predictions.shape[0]
    assert b % N_CORES == 0
    n_rows = b // N_CORES
    nc = _get_nc(n_rows, n_tiles, use_act)
    preds = np.ascontiguousarray(predictions, dtype=np.float32).reshape(
        N_CORES, n_rows, 3
    )
    targs = np.ascontiguousarray(targets, dtype=np.float32).reshape(
        N_CORES, n_rows, 3
    )
    in_maps = [
        {"pt": np.stack([preds[c], targs[c]])} for c in range(N_CORES)
    ]
    res = run_bass_kernel_spmd(nc, in_maps, list(range(N_CORES)), **run_kwargs)
    partials = np.stack([r["partials"] for r in res.results])
    return partials, res


# ---------------------------------------------------------------------------
# Entry point
# ---------------------------------------------------------------------------


def kernel(predictions: np.ndarray, targets: np.ndarray, sigma: np.ndarray) -> np.ndarray:
    predictions = np.asarray(predictions, dtype=np.float32)
    targets = np.asarray(targets, dtype=np.float32)
    sigma64 = np.asarray(sigma, dtype=np.float64)

    _, logdet = np.linalg.slogdet(sigma64)
    n = predictions.shape[0]

    if _sigma_is_scalar_identity(sigma64) and n == B_FULL:
        sums, _ = run_device_sumsq_v3(predictions, targets)
        total = sumsq_from_cols_v3(sums)
        mean_mahal = total / (sigma64[0, 0] * n)
    else:
        partials, _ = run_device_partials(predictions, targets, n_tiles=32)
        g = gram_from_partials(partials)
        sigma_inv = np.linalg.inv(sigma64)
        mean_mahal = float((sigma_inv * g).sum()) / n

    loss = abs(logdet + mean_mahal)
    return np.float32(loss)
